# revision 1
# baseline (speedup 1.0000x reference)
"""Trainium2 Bass kernel for nn_Attention_91293824844283.

Multi-head attention (identity rep): per-head 1x1-conv Q/K/V projections,
softmax(Q K^T / sqrt(E)) V, per-head output projection summed over heads.

Shapes: B=4, N=2048, D=512, H=8, E=64.

Sharding over 8 cores: core c -> (batch b = c//2, head-group g = c%2 of 4
heads). Each core computes the partial output sum over its 4 heads for its
batch; host adds the two partials per batch.

Device-side layout/algorithm (per core):
  - Host supplies x2[b].T, x1[b].T, v[b].T (bf16) plus packed transposed
    weights. Scale 1/sqrt(E) is folded into Wq.
  - Q^T/K^T [E,N] computed per head-pair (2x64 rows packed into 128
    partitions).
  - V [N, 4*66] with a ones column per head (slot width 66) so the PV
    matmul also produces the softmax denominators (M=65).
  - Attention runs per (pair, nq-quarter): the two heads' S^T matmuls
    (K=64) land in disjoint PE row groups (partitions 0:64 / 64:128) and
    run concurrently; both heads' scores share one [128,1024] PSUM tile
    so a single ACT exp op serves the pair. PV accumulates each head's
    rep~^T [65, 512] in PSUM over the 16 nk tiles.
  - rep~^T -> SBUF, PE-transpose 128-col chunks, reciprocal of the sums
    column, per-partition scale (DVE), PE-transpose back to rep^T (bf16).
  - Pair-1 Q/K projections are emitted after pair-0 attention so the
    scheduler uses them as PE gap filler (2 spare PSUM banks).
  - Output projection: out[nq,D] += rep^T.T @ Wo^T accumulated over the 4
    heads in PSUM, staged to SBUF, DMA'd to DRAM fp32.
"""

import numpy as np
import ml_dtypes
from contextlib import ExitStack

B, N, D, H, E = 4, 2048, 512, 8, 64
HPC = 4            # heads per core
N_CORES = 8
NKT = N // 128     # 16 nk tiles
VSLOT = 66         # V slot: 64 V cols + 1 ones col + 1 pad
KT = D // 128      # 4 contraction tiles for projections
QW = 512           # nq quarter width

_CACHE = {}


def _build():
    import concourse.tile as tile
    from concourse import bacc, mybir

    bf16 = mybir.dt.bfloat16
    f32 = mybir.dt.float32
    Exp = mybir.ActivationFunctionType.Exp

    nc = bacc.Bacc(
        "TRN2", target_bir_lowering=False, debug=False, num_devices=N_CORES
    )
    xqT = nc.dram_tensor("xqT", [KT, 128, N], bf16, kind="ExternalInput").ap()
    xkT = nc.dram_tensor("xkT", [KT, 128, N], bf16, kind="ExternalInput").ap()
    vT = nc.dram_tensor("vT", [KT, 128, N], bf16, kind="ExternalInput").ap()
    wqT = nc.dram_tensor("wqT", [2, KT, 128, 128], bf16, kind="ExternalInput").ap()
    wkT = nc.dram_tensor("wkT", [2, KT, 128, 128], bf16, kind="ExternalInput").ap()
    wvT = nc.dram_tensor("wvT", [KT, 128, HPC * E], bf16, kind="ExternalInput").ap()
    woT = nc.dram_tensor("woT", [HPC, E, D], bf16, kind="ExternalInput").ap()
    identf = nc.dram_tensor("identf", [128, 128], f32, kind="ExternalInput").ap()
    identb = nc.dram_tensor("identb", [128, 128], bf16, kind="ExternalInput").ap()
    outp = nc.dram_tensor("outp", [NKT, 128, D], f32, kind="ExternalOutput").ap()

    with tile.TileContext(nc) as tc, ExitStack() as ctx:
        cp = ctx.enter_context(tc.tile_pool(name="const", bufs=1))

        # --- persistent SBUF tiles ---
        xq = [cp.tile([128, N], bf16, tag=f"xq{k}", name=f"xq{k}") for k in range(KT)]
        xk = [cp.tile([128, N], bf16, tag=f"xk{k}", name=f"xk{k}") for k in range(KT)]
        xv = [cp.tile([128, N], bf16, tag=f"xv{k}", name=f"xv{k}") for k in range(KT)]
        wq = [[cp.tile([128, 128], bf16, tag=f"wq{p}{k}", name=f"wq{p}{k}")
               for k in range(KT)] for p in range(2)]
        wk = [[cp.tile([128, 128], bf16, tag=f"wk{p}{k}", name=f"wk{p}{k}")
               for k in range(KT)] for p in range(2)]
        wv = [cp.tile([128, HPC * E], bf16, tag=f"wv{k}", name=f"wv{k}")
              for k in range(KT)]
        wo = [cp.tile([E, D], bf16, tag=f"wo{h}", name=f"wo{h}") for h in range(HPC)]
        idf = cp.tile([128, 128], f32, tag="idf")
        qt = [cp.tile([128, N], bf16, tag=f"qt{p}", name=f"qt{p}") for p in range(2)]
        kt = [cp.tile([128, N], bf16, tag=f"kt{p}", name=f"kt{p}") for p in range(2)]
        vaug = [cp.tile([128, HPC * VSLOT], bf16, tag=f"va{t}", name=f"va{t}")
                for t in range(NKT)]
        repbf16 = [cp.tile([E, N], bf16, tag=f"rb{h}", name=f"rb{h}")
                   for h in range(HPC)]
        idb = cp.tile([128, 128], bf16, tag="idb")

        # --- input DMAs, chunked 512 columns at a time so the first
        # projection matmuls can start after ~1/4 of the data has landed.
        # K path first (attention quarter 0 sweeps all of K but needs only
        # the first Q quarter), then V, then the remaining Q quarters.
        de = [nc.sync, nc.scalar]   # both HWDGE-capable queue sets
        for k in range(KT):
            for p in range(2):
                de[k % 2].dma_start(wq[p][k][:], wqT[p, k])
                de[(k + 1) % 2].dma_start(wk[p][k][:], wkT[p, k])
            de[k % 2].dma_start(wv[k][:], wvT[k])
        for c in range(4):
            sl = slice(c * 512, (c + 1) * 512)
            for k in range(KT):
                de[k % 2].dma_start(xk[k][:, sl], xkT[k][:, sl])
            if c == 0:
                for k in range(KT):
                    de[(k + 1) % 2].dma_start(xq[k][:, sl], xqT[k][:, sl])
        for c in range(4):
            sl = slice(c * 512, (c + 1) * 512)
            for k in range(KT):
                de[k % 2].dma_start(xv[k][:, sl], vT[k][:, sl])
        for c in range(1, 4):
            sl = slice(c * 512, (c + 1) * 512)
            for k in range(KT):
                de[(k + 1) % 2].dma_start(xq[k][:, sl], xqT[k][:, sl])
        for h in range(HPC):
            nc.sync.dma_start(wo[h][:], woT[h])
        nc.sync.dma_start(idf[:], identf[:])
        nc.sync.dma_start(idb[:], identb[:])

        # --- PE warmup burst: dependency-free dummy matmuls fill the DMA
        # window and push HAM to K=8/8 before the first projection.
        warm_sb = cp.tile([128, 512], bf16, tag="warm_sb")
        nc.gpsimd.memset(warm_sb[:], 0.0)
        with tc.tile_pool(name="warmps", bufs=1, space="PSUM") as wps:
            wpt = wps.tile([128, 512], f32, tag="w", name="warm_ps")
            for i in range(32):
                nc.tensor.matmul(wpt[:], warm_sb[:, 0:128], warm_sb[:],
                                 start=True, stop=True)

        def proj_chunk(pool, dst, w, x, c, tag="proj"):
            ps = pool.tile([128, 512], f32, tag=tag, name="proj_ps")
            sl = slice(c * 512, (c + 1) * 512)
            for k in range(KT):
                nc.tensor.matmul(
                    ps[:], w[k][:], x[k][:, sl],
                    start=(k == 0), stop=(k == KT - 1),
                )
            nc.vector.tensor_copy(dst[:, sl], ps[:])

        def qk_proj(pool, p, tag="proj"):
            for c in range(4):
                proj_chunk(pool, kt[p], wk[p], xk, c, tag)
            for c in range(4):
                proj_chunk(pool, qt[p], wq[p], xq, c, tag)

        # --- attention pools (created before projections: a closing
        # projection pool would barrier attention PSUM allocation behind
        # ALL upfront work; instead projections share the fill pool).
        # PSUM: s pair tile 2 banks x bufs=2 + rep 2x1 bank + fill 2x1 = 8.
        sp = ctx.enter_context(tc.tile_pool(name="spsum", bufs=2, space="PSUM"))
        rp = ctx.enter_context(tc.tile_pool(name="rpsum", bufs=1, space="PSUM"))
        fpp = ctx.enter_context(tc.tile_pool(name="fill", bufs=2, space="PSUM"))
        ptp = ctx.enter_context(tc.tile_pool(name="ptile", bufs=4))
        smp = ctx.enter_context(tc.tile_pool(name="small", bufs=6))

        # --- upfront projections, ordered to unblock attention quarter 0:
        # K pair-0 (all chunks) + Q pair-0 chunk 0, then V, then Q rest.
        for c in range(4):
            proj_chunk(fpp, kt[0], wk[0], xk, c, tag="f")
        proj_chunk(fpp, qt[0], wq[0], xq, 0, tag="f")
        for t in range(NKT):
            nc.gpsimd.memset(vaug[t][:], 1.0)
            ps = fpp.tile([128, HPC * E], f32, tag="f", name="vproj_ps")
            tsl = slice(t * 128, (t + 1) * 128)
            for k in range(KT):
                nc.tensor.matmul(
                    ps[:], xv[k][:, tsl], wv[k][:],
                    start=(k == 0), stop=(k == KT - 1),
                )
            for h in range(HPC):
                nc.vector.tensor_copy(
                    vaug[t][:, h * VSLOT:h * VSLOT + E],
                    ps[:, h * E:(h + 1) * E],
                )
        for c in range(1, 4):
            proj_chunk(fpp, qt[0], wq[0], xq, c, tag="f")

        def attention_pair(p, after_quarter=None, per_chunk=None):
            for q4 in range(4):
                qoff = q4 * QW
                rep = [
                    rp.tile([65, QW], f32, tag=f"rep{s}", name=f"rep{s}")
                    for s in range(2)
                ]
                for t in range(NKT):
                    tsl = slice(t * 128, (t + 1) * 128)
                    spair = sp.tile([128, 2 * QW], f32, tag="s", name="spair")
                    for s in range(2):
                        esl = slice(s * 64, (s + 1) * 64)
                        nc.tensor.matmul(
                            spair[:, s * QW:(s + 1) * QW],
                            kt[p][esl, tsl], qt[p][esl, qoff:qoff + QW],
                            start=True, stop=True,
                        )
                    pt = ptp.tile([128, 2 * QW], bf16, tag="p", name="pt")
                    nc.scalar.activation(pt[:], spair[:], Exp)
                    for s in range(2):
                        h = 2 * p + s
                        vsl = slice(h * VSLOT, h * VSLOT + 65)
                        nc.tensor.matmul(
                            rep[s][:],
                            vaug[t][:, vsl], pt[:, s * QW:(s + 1) * QW],
                            start=(t == 0), stop=(t == NKT - 1),
                        )
                # drain + normalize (transpose, scale rows, transpose back);
                # chunk-major so a per-chunk hook can interleave output work
                rts = {}
                for s in range(2):
                    rts[s] = smp.tile([65, QW], f32, tag=f"rts{s}", name=f"rts{s}")
                    nc.vector.tensor_copy(rts[s][:], rep[s][:])
                for tt in range(QW // 128):
                    csl = slice(tt * 128, (tt + 1) * 128)
                    osl = slice(qoff + tt * 128, qoff + (tt + 1) * 128)
                    for s in range(2):
                        h = 2 * p + s
                        tr1 = fpp.tile([128, 65], f32, tag="f", name=f"tr1_{s}")
                        nc.tensor.transpose(tr1[:], rts[s][:, csl], idf[0:65, 0:65])
                        r = smp.tile([128, 1], f32, tag="r")
                        nc.vector.reciprocal(r[:], tr1[:, 64:65])
                        rb = smp.tile([128, E], bf16, tag="rb")
                        nc.vector.tensor_scalar_mul(rb[:], tr1[:, 0:E], r[:])
                        tr2 = fpp.tile([E, 128], bf16, tag="f", name=f"tr2_{s}")
                        nc.tensor.transpose(tr2[:], rb[:], idb[:])
                        nc.vector.tensor_copy(repbf16[h][:, osl], tr2[:])
                    if per_chunk is not None:
                        per_chunk(4 * q4 + tt)
                if after_quarter is not None:
                    after_quarter(q4)

        def outproj_tile(t):
            # out tile t (all 4 heads' rep^T for this tile ready by now)
            tsl = slice(t * 128, (t + 1) * 128)
            ops = fpp.tile([128, D], f32, tag="f", name="ops")
            for h in range(HPC):
                nc.tensor.matmul(
                    ops[:], repbf16[h][:, tsl], wo[h][:],
                    start=(h == 0), stop=(h == HPC - 1),
                )
            ost = ptp.tile([128, D], f32, tag="ost")
            nc.vector.tensor_copy(ost[:], ops[:])
            nc.sync.dma_start(outp[t], ost[:])

        # pair-1 projections spread across pair-0's quarters: each
        # quarter's tail emits two chunks, so the scheduler drains them in
        # PE gaps well before pair-1 attention needs them.
        def pair1_proj_part(q4):
            if q4 < 2:
                proj_chunk(fpp, kt[1], wk[1], xk, 2 * q4, tag="f")
                proj_chunk(fpp, kt[1], wk[1], xk, 2 * q4 + 1, tag="f")
            else:
                proj_chunk(fpp, qt[1], wq[1], xq, 2 * (q4 - 2), tag="f")
                proj_chunk(fpp, qt[1], wq[1], xq, 2 * (q4 - 2) + 1, tag="f")

        attention_pair(0, after_quarter=pair1_proj_part)
        attention_pair(1, per_chunk=outproj_tile)

    nc.compile()
    return nc


def _prep_core_inputs(c, x1, x2, v, Wq, Wk, Wv, Wo, identf, identb):
    bf = ml_dtypes.bfloat16
    b, g = c // 2, c % 2
    hs = slice(g * HPC, (g + 1) * HPC)
    wq = (Wq[hs] * (1.0 / np.sqrt(E))).astype(np.float32)   # fold 1/sqrt(E)
    wk, wv, wo = Wk[hs], Wv[hs], Wo[hs]

    def t_pack_pair(w):
        # [4,E,D] -> per pair p: concat(w[2p].T, w[2p+1].T, axis=1) [D,128]
        out = np.empty((2, KT, 128, 128), bf)
        for p in range(2):
            m = np.concatenate([w[2 * p].T, w[2 * p + 1].T], axis=1)  # [D,128]
            out[p] = m.reshape(KT, 128, 128).astype(bf)
        return out

    xq = np.ascontiguousarray(x2[b].T).astype(bf).reshape(KT, 128, N)
    xk = np.ascontiguousarray(x1[b].T).astype(bf).reshape(KT, 128, N)
    xv = np.ascontiguousarray(v[b].T).astype(bf).reshape(KT, 128, N)
    wvT = np.concatenate([wv[h].T for h in range(HPC)], axis=1)  # [D, 256]
    woT = np.stack([wo[h].T for h in range(HPC)])                # [4, E, D]
    return {
        "xqT": xq, "xkT": xk, "vT": xv,
        "wqT": t_pack_pair(wq), "wkT": t_pack_pair(wk),
        "wvT": np.ascontiguousarray(wvT).astype(bf).reshape(KT, 128, HPC * E),
        "woT": woT.astype(bf),
        "identf": identf, "identb": identb,
    }


def kernel(**inputs):
    from concourse.bass_utils import run_bass_kernel_spmd

    x1 = np.asarray(inputs["x1"], np.float32)
    x2 = np.asarray(inputs["x2"], np.float32)
    v = np.asarray(inputs["v"], np.float32)
    Wq = np.asarray(inputs["Wq"], np.float32)
    Wk = np.asarray(inputs["Wk"], np.float32)
    Wv = np.asarray(inputs["Wv"], np.float32)
    Wo = np.asarray(inputs["Wo"], np.float32)

    if "nc" not in _CACHE:
        _CACHE["nc"] = _build()
    nc = _CACHE["nc"]

    identf = np.eye(128, dtype=np.float32)
    identb = np.eye(128, dtype=ml_dtypes.bfloat16)
    in_maps = [
        _prep_core_inputs(c, x1, x2, v, Wq, Wk, Wv, Wo, identf, identb)
        for c in range(N_CORES)
    ]
    res = run_bass_kernel_spmd(nc, in_maps, list(range(N_CORES)))
    out = np.empty((B, N, D), np.float32)
    for b in range(B):
        out[b] = (
            res.results[2 * b]["outp"].reshape(N, D)
            + res.results[2 * b + 1]["outp"].reshape(N, D)
        )
    return out



# revision 5
# speedup vs baseline: 1.0644x; 1.0644x over previous
"""Trainium2 Bass kernel for nn_Attention_91293824844283.

Multi-head attention (identity rep): per-head 1x1-conv Q/K/V projections,
softmax(Q K^T / sqrt(E)) V, per-head output projection summed over heads.

Shapes: B=4, N=2048, D=512, H=8, E=64.

Sharding over 8 cores: core c -> (batch b = c//2, head-group g = c%2 of 4
heads). Each core computes the partial output sum over its 4 heads for its
batch; host adds the two partials per batch.

Device-side pipeline (per core), ScalarE(exp)-paced:
  - Q^T/K^T [E,N] per head-pair (2x64 rows packed into 128 partitions),
    1/sqrt(E) folded into Wq. V [N, 4 slots of 66] with a ones column per
    head so the PV matmul also produces the softmax denominators (M=65).
  - Attention per (pair, nq-quarter, nk-tile): two heads' S^T matmuls
    (K=64) in disjoint PE row groups share one [128,1024] PSUM tile; one
    ACT exp per tile-pair; PV accumulates rep~^T [65, 512] over 16 tiles.
  - Normalization without PE transposes: d sits in rep row 64; DVE
    reciprocal -> K=1 PE broadcast matmul -> DVE multiply fused into the
    PSUM->SBUF drain of rep (all partition-aligned).
  - Output projection: 4 K=64 matmuls accumulate out[nq,D] in PSUM,
    cast to bf16, DMA out. Host sums the two half-head partials in f32.
  - Emission interleaves projections/outproj into PE gaps so the exp
    stream (the critical engine at ~1.33us per [128,1024] tile) never
    starves; input DMAs ride sync (+scalar only before the first exp).
"""

import numpy as np
import ml_dtypes
from contextlib import ExitStack

B, N, D, H, E = 4, 2048, 512, 8, 64
HPC = 4            # heads per core
N_CORES = 8
NKT = N // 128     # 16 nk tiles
VSLOT = 66         # V slot: 64 V cols + 1 ones col + 1 pad
KT = D // 128      # 4 contraction tiles for projections
QW = 512           # nq quarter width

_CACHE = {}


def _build():
    import concourse.tile as tile
    from concourse import bacc, mybir

    bf16 = mybir.dt.bfloat16
    f32 = mybir.dt.float32
    Exp = mybir.ActivationFunctionType.Exp

    nc = bacc.Bacc(
        "TRN2", target_bir_lowering=False, debug=False, num_devices=N_CORES
    )
    xqT = nc.dram_tensor("xqT", [KT, 128, N], bf16, kind="ExternalInput").ap()
    xkT = nc.dram_tensor("xkT", [KT, 128, N], bf16, kind="ExternalInput").ap()
    vT = nc.dram_tensor("vT", [KT, 128, N], bf16, kind="ExternalInput").ap()
    wqT = nc.dram_tensor("wqT", [2, KT, 128, 128], bf16, kind="ExternalInput").ap()
    wkT = nc.dram_tensor("wkT", [2, KT, 128, 128], bf16, kind="ExternalInput").ap()
    wvT = nc.dram_tensor("wvT", [KT, 128, HPC * E], bf16, kind="ExternalInput").ap()
    woT = nc.dram_tensor("woT", [HPC, E, D], bf16, kind="ExternalInput").ap()
    outp = nc.dram_tensor("outp", [NKT, 128, D], bf16, kind="ExternalOutput").ap()

    with tile.TileContext(nc) as tc, ExitStack() as ctx:
        cp = ctx.enter_context(tc.tile_pool(name="const", bufs=1))

        # --- persistent SBUF tiles ---
        xq = [cp.tile([128, N], bf16, tag=f"xq{k}", name=f"xq{k}") for k in range(KT)]
        xk = [cp.tile([128, N], bf16, tag=f"xk{k}", name=f"xk{k}") for k in range(KT)]
        xv = [cp.tile([128, N], bf16, tag=f"xv{k}", name=f"xv{k}") for k in range(KT)]
        wq = [[cp.tile([128, 128], bf16, tag=f"wq{p}{k}", name=f"wq{p}{k}")
               for k in range(KT)] for p in range(2)]
        wk = [[cp.tile([128, 128], bf16, tag=f"wk{p}{k}", name=f"wk{p}{k}")
               for k in range(KT)] for p in range(2)]
        wv = [cp.tile([128, HPC * E], bf16, tag=f"wv{k}", name=f"wv{k}")
              for k in range(KT)]
        wo = [cp.tile([E, D], bf16, tag=f"wo{h}", name=f"wo{h}") for h in range(HPC)]
        qt = [cp.tile([128, N], bf16, tag=f"qt{p}", name=f"qt{p}") for p in range(2)]
        kt = [cp.tile([128, N], bf16, tag=f"kt{p}", name=f"kt{p}") for p in range(2)]
        vaug = [cp.tile([128, HPC, VSLOT], bf16, tag=f"va{t}", name=f"va{t}")
                for t in range(NKT)]
        repbf16 = [cp.tile([E, N], bf16, tag=f"rb{h}", name=f"rb{h}")
                   for h in range(HPC)]
        onesb = cp.tile([65, E], bf16, tag="onesb")

        # --- input DMAs. Head-critical tensors split across the sync and
        # scalar HWDGE queues (scalar is free until the first exp); the
        # long tail rides sync so ScalarE stays exp-only.
        nc.gpsimd.memset(onesb[:], 1.0)
        for k in range(KT):
            nc.sync.dma_start(wk[0][k][:], wkT[0, k])
            nc.scalar.dma_start(wq[0][k][:], wqT[0, k])
        c0 = slice(0, 512)
        for k in range(KT):
            nc.sync.dma_start(xk[k][:, c0], xkT[k][:, c0])
            nc.scalar.dma_start(xq[k][:, c0], xqT[k][:, c0])
        for k in range(KT):
            nc.scalar.dma_start(wv[k][:], wvT[k])
            nc.scalar.dma_start(xv[k][:, c0], vT[k][:, c0])
        for c in range(1, 4):
            sl = slice(c * 512, (c + 1) * 512)
            for k in range(KT):
                nc.sync.dma_start(xk[k][:, sl], xkT[k][:, sl])
            for k in range(KT):
                nc.sync.dma_start(xv[k][:, sl], vT[k][:, sl])
            for k in range(KT):
                nc.sync.dma_start(xq[k][:, sl], xqT[k][:, sl])
        for k in range(KT):
            nc.sync.dma_start(wk[1][k][:], wkT[1, k])
            nc.sync.dma_start(wq[1][k][:], wqT[1, k])
        for h in range(HPC):
            nc.sync.dma_start(wo[h][:], woT[h])

        # --- PE warmup burst: dependency-free dummy matmuls fill the DMA
        # window and push HAM to K=8/8 before the first projection.
        warm_sb = cp.tile([128, 512], bf16, tag="warm_sb")
        nc.gpsimd.memset(warm_sb[:], 0.0)
        with tc.tile_pool(name="warmps", bufs=1, space="PSUM") as wps:
            wpt = wps.tile([128, 512], f32, tag="w", name="warm_ps")
            for i in range(16):
                nc.tensor.matmul(wpt[:], warm_sb[:, 0:128], warm_sb[:],
                                 start=True, stop=True)

        # --- PSUM pools: spair 2 banks x 2 bufs + rep 2 x 1 bank + fill
        # 2 x 1 bank = 8 banks.
        sp = ctx.enter_context(tc.tile_pool(name="spsum", bufs=2, space="PSUM"))
        rp = ctx.enter_context(tc.tile_pool(name="rpsum", bufs=1, space="PSUM"))
        fpp = ctx.enter_context(tc.tile_pool(name="fill", bufs=2, space="PSUM"))
        ptp = ctx.enter_context(tc.tile_pool(name="ptile", bufs=4))
        smp = ctx.enter_context(tc.tile_pool(name="small", bufs=6))

        def proj_chunk(dst, w, x, c, tag="f"):
            ps = fpp.tile([128, 512], f32, tag=tag, name="proj_ps")
            sl = slice(c * 512, (c + 1) * 512)
            for k in range(KT):
                nc.tensor.matmul(
                    ps[:], w[k][:], x[k][:, sl],
                    start=(k == 0), stop=(k == KT - 1),
                )
            nc.vector.tensor_copy(dst[:, sl], ps[:])

        def vproj_tile(t):
            nc.gpsimd.memset(vaug[t][:], 1.0)
            ps = fpp.tile([128, HPC * E], f32, tag="f", name="vproj_ps")
            tsl = slice(t * 128, (t + 1) * 128)
            for k in range(KT):
                nc.tensor.matmul(
                    ps[:], xv[k][:, tsl], wv[k][:],
                    start=(k == 0), stop=(k == KT - 1),
                )
            nc.vector.tensor_copy(vaug[t][:, :, 0:E], ps[:])

        # --- minimal head: just enough projection for attention (p0, q0).
        proj_chunk(kt[0], wk[0], xk, 0)
        proj_chunk(qt[0], wq[0], xq, 0)
        for t in range(4):
            vproj_tile(t)

        def normalize(p, q4):
            # rep_ps[s] rows 0:64 = unnormalized rep^T, row 64 = denom d.
            # dinv row -> K=1 matmul broadcast over 64 partitions -> fused
            # scale during the PSUM->SBUF drain. All ops partition-aligned.
            qoff = q4 * QW
            for s in range(2):
                h = 2 * p + s
                dinvs = smp.tile([65, QW], bf16, tag=f"di{s}", name=f"di{s}")
                with nc.allow_low_precision(reason="bf16 softmax denom"):
                    nc.vector.reciprocal(dinvs[64:65, :], rep_ps[s][64:65, :])
                dbp = fpp.tile([E, QW], f32, tag="f", name=f"dbp{s}")
                nc.tensor.matmul(dbp[:], onesb[64:65, :], dinvs[64:65, :],
                                 start=True, stop=True)
                dbs = smp.tile([E, QW], f32, tag=f"db{s}", name=f"db{s}")
                nc.vector.tensor_copy(dbs[:], dbp[:])
                nc.vector.tensor_mul(
                    repbf16[h][:, qoff:qoff + QW], rep_ps[s][0:E, :], dbs[:]
                )

        rep_ps = None

        def attention_quarter(p, q4, hooks=()):
            # hooks: list of (tile_idx, fn) run after that nk tile's emission
            nonlocal rep_ps
            qoff = q4 * QW
            rep_ps = [
                rp.tile([65, QW], f32, tag=f"rep{s}", name=f"rep{s}")
                for s in range(2)
            ]
            hd = dict()
            for t, fn in hooks:
                hd.setdefault(t, []).append(fn)
            for t in range(NKT):
                tsl = slice(t * 128, (t + 1) * 128)
                spair = sp.tile([128, 2 * QW], f32, tag="s", name="spair")
                for s in range(2):
                    esl = slice(s * 64, (s + 1) * 64)
                    nc.tensor.matmul(
                        spair[:, s * QW:(s + 1) * QW],
                        kt[p][esl, tsl], qt[p][esl, qoff:qoff + QW],
                        start=True, stop=True,
                    )
                pt = ptp.tile([128, 2 * QW], bf16, tag="p", name="pt")
                nc.scalar.activation(pt[:], spair[:], Exp)
                for s in range(2):
                    h = 2 * p + s
                    nc.tensor.matmul(
                        rep_ps[s][:],
                        vaug[t][:, h, 0:65], pt[:, s * QW:(s + 1) * QW],
                        start=(t == 0), stop=(t == NKT - 1),
                    )
                for fn in hd.get(t, ()):
                    fn()
            normalize(p, q4)

        def outproj_tile(t):
            tsl = slice(t * 128, (t + 1) * 128)
            ops = fpp.tile([128, D], f32, tag="f", name="ops")
            for h in range(HPC):
                nc.tensor.matmul(
                    ops[:], repbf16[h][:, tsl], wo[h][:],
                    start=(h == 0), stop=(h == HPC - 1),
                )
            ost = ptp.tile([128, D], bf16, tag="ost")
            nc.vector.tensor_copy(ost[:], ops[:])
            nc.sync.dma_start(outp[t], ost[:])

        # --- pair 0: remaining projections interleaved into PE gaps.
        # Every producer is EMITTED before its consumer (quarter 0 sweeps all
        # 16 nk tiles, so every vaug tile lands just-in-time inside q0).
        attention_quarter(0, 0, hooks=[
            (0, lambda: vproj_tile(4)), (1, lambda: vproj_tile(5)),
            (2, lambda: vproj_tile(6)),
            (3, lambda: proj_chunk(kt[0], wk[0], xk, 1)),
            (3, lambda: vproj_tile(7)),
            (4, lambda: vproj_tile(8)), (5, lambda: vproj_tile(9)),
            (6, lambda: vproj_tile(10)),
            (7, lambda: proj_chunk(kt[0], wk[0], xk, 2)),
            (7, lambda: vproj_tile(11)),
            (8, lambda: vproj_tile(12)), (9, lambda: vproj_tile(13)),
            (10, lambda: vproj_tile(14)),
            (11, lambda: proj_chunk(kt[0], wk[0], xk, 3)),
            (11, lambda: vproj_tile(15)),
            (15, lambda: proj_chunk(qt[0], wq[0], xq, 1)),
        ])
        attention_quarter(0, 1, hooks=[
            (6, lambda: proj_chunk(kt[1], wk[1], xk, 0)),
            (9, lambda: proj_chunk(kt[1], wk[1], xk, 1)),
            (12, lambda: proj_chunk(qt[0], wq[0], xq, 2)),
        ])
        attention_quarter(0, 2, hooks=[
            (2, lambda: proj_chunk(kt[1], wk[1], xk, 2)),
            (5, lambda: proj_chunk(kt[1], wk[1], xk, 3)),
            (8, lambda: proj_chunk(qt[1], wq[1], xq, 0)),
            (12, lambda: proj_chunk(qt[0], wq[0], xq, 3)),
        ])
        attention_quarter(0, 3, hooks=[
            (2, lambda: proj_chunk(qt[1], wq[1], xq, 1)),
            (6, lambda: proj_chunk(qt[1], wq[1], xq, 2)),
            (10, lambda: proj_chunk(qt[1], wq[1], xq, 3)),
        ])
        # --- pair 1: output projection per nq tile as soon as its quarter
        # is normalized.
        for q4 in range(4):
            attention_quarter(1, q4)
            for t in range(4 * q4, 4 * q4 + 4):
                outproj_tile(t)

    nc.compile()
    return nc


def _prep_core_inputs(c, x1, x2, v, Wq, Wk, Wv, Wo, identf=None, identb=None):
    bf = ml_dtypes.bfloat16
    b, g = c // 2, c % 2
    hs = slice(g * HPC, (g + 1) * HPC)
    wq = (Wq[hs] * (1.0 / np.sqrt(E))).astype(np.float32)   # fold 1/sqrt(E)
    wk, wv, wo = Wk[hs], Wv[hs], Wo[hs]

    def t_pack_pair(w):
        # [4,E,D] -> per pair p: concat(w[2p].T, w[2p+1].T, axis=1) [D,128]
        out = np.empty((2, KT, 128, 128), bf)
        for p in range(2):
            m = np.concatenate([w[2 * p].T, w[2 * p + 1].T], axis=1)  # [D,128]
            out[p] = m.reshape(KT, 128, 128).astype(bf)
        return out

    xq = np.ascontiguousarray(x2[b].T).astype(bf).reshape(KT, 128, N)
    xk = np.ascontiguousarray(x1[b].T).astype(bf).reshape(KT, 128, N)
    xv = np.ascontiguousarray(v[b].T).astype(bf).reshape(KT, 128, N)
    wvT = np.concatenate([wv[h].T for h in range(HPC)], axis=1)  # [D, 256]
    woT = np.stack([wo[h].T for h in range(HPC)])                # [4, E, D]
    return {
        "xqT": xq, "xkT": xk, "vT": xv,
        "wqT": t_pack_pair(wq), "wkT": t_pack_pair(wk),
        "wvT": np.ascontiguousarray(wvT).astype(bf).reshape(KT, 128, HPC * E),
        "woT": woT.astype(bf),
    }


def kernel(**inputs):
    from concourse.bass_utils import run_bass_kernel_spmd

    x1 = np.asarray(inputs["x1"], np.float32)
    x2 = np.asarray(inputs["x2"], np.float32)
    v = np.asarray(inputs["v"], np.float32)
    Wq = np.asarray(inputs["Wq"], np.float32)
    Wk = np.asarray(inputs["Wk"], np.float32)
    Wv = np.asarray(inputs["Wv"], np.float32)
    Wo = np.asarray(inputs["Wo"], np.float32)

    if "nc" not in _CACHE:
        _CACHE["nc"] = _build()
    nc = _CACHE["nc"]

    in_maps = [
        _prep_core_inputs(c, x1, x2, v, Wq, Wk, Wv, Wo)
        for c in range(N_CORES)
    ]
    res = run_bass_kernel_spmd(nc, in_maps, list(range(N_CORES)))
    out = np.empty((B, N, D), np.float32)
    for b in range(B):
        out[b] = (
            res.results[2 * b]["outp"].reshape(N, D).astype(np.float32)
            + res.results[2 * b + 1]["outp"].reshape(N, D).astype(np.float32)
        )
    return out


# revision 7
# speedup vs baseline: 1.2898x; 1.2118x over previous
"""Trainium2 Bass kernel for nn_Attention_91293824844283.

Multi-head attention (identity rep): per-head 1x1-conv Q/K/V projections,
softmax(Q K^T / sqrt(E)) V, per-head output projection summed over heads.

Shapes: B=4, N=2048, D=512, H=8, E=64.

Sharding over 8 cores: core c -> (batch b = c//2, head-group g = c%2 of 4
heads). Each core computes the partial output sum over its 4 heads for its
batch; host adds the two partials per batch.

Device-side pipeline (per core), ScalarE(exp)-paced at ~1.1us per
[128,1024] tile (128 tiles ~ 142us of exp is the hard floor; everything
else hides under it):
  - Q^T/K^T [E,N] per head-pair (2x64 rows in 128 partitions), 1/sqrt(E)
    folded into Wq. V [N, 4 slots of 66] with a ones column per head so
    the PV matmul also produces the softmax denominators (M=65).
  - Inner loop software-pipelined one tile ahead (S(t+1) is emitted
    before PV(t)) so a stalled PV never blocks the S -> exp chain on the
    in-order PE queue.
  - Normalization without PE transposes: rep PSUM is drained immediately
    (bf16 copy frees the banks), then a deferred chain in the NEXT
    quarter does d-row K=1 broadcast matmul -> reciprocal_approx_fast ->
    fused scale (all partition-aligned, off the critical path).
  - Output projection: 4 K=64 matmuls accumulate out[nq,D] in PSUM,
    cast to bf16, DMA out. Host sums the two half-head partials in f32.
  - DMAs are deadline-ordered; the scalar HWDGE queue only carries head
    loads that finish before the first exp.
"""

import numpy as np
import ml_dtypes
from contextlib import ExitStack

B, N, D, H, E = 4, 2048, 512, 8, 64
HPC = 4            # heads per core
N_CORES = 8
NKT = N // 128     # 16 nk tiles
VSLOT = 66         # V slot: 64 V cols + 1 ones col + 1 pad
KT = D // 128      # 4 contraction tiles for projections
QW = 512           # nq quarter width

_CACHE = {}


def _build():
    import concourse.tile as tile
    from concourse import bacc, mybir

    bf16 = mybir.dt.bfloat16
    f32 = mybir.dt.float32
    Exp = mybir.ActivationFunctionType.Exp

    nc = bacc.Bacc(
        "TRN2", target_bir_lowering=False, debug=False, num_devices=N_CORES
    )
    xqT = nc.dram_tensor("xqT", [KT, 128, N], bf16, kind="ExternalInput").ap()
    xkT = nc.dram_tensor("xkT", [KT, 128, N], bf16, kind="ExternalInput").ap()
    vT = nc.dram_tensor("vT", [KT, 128, N], bf16, kind="ExternalInput").ap()
    wqT = nc.dram_tensor("wqT", [2, KT, 128, 128], bf16, kind="ExternalInput").ap()
    wkT = nc.dram_tensor("wkT", [2, KT, 128, 128], bf16, kind="ExternalInput").ap()
    wvT = nc.dram_tensor("wvT", [KT, 128, HPC * E], bf16, kind="ExternalInput").ap()
    woT = nc.dram_tensor("woT", [HPC, E, D], bf16, kind="ExternalInput").ap()
    outp = nc.dram_tensor("outp", [NKT, 128, D], bf16, kind="ExternalOutput").ap()

    with tile.TileContext(nc) as tc, ExitStack() as ctx:
        cp = ctx.enter_context(tc.tile_pool(name="const", bufs=1))

        # --- persistent SBUF tiles ---
        xq = [cp.tile([128, N], bf16, tag=f"xq{k}", name=f"xq{k}") for k in range(KT)]
        xk = [cp.tile([128, N], bf16, tag=f"xk{k}", name=f"xk{k}") for k in range(KT)]
        xv = [cp.tile([128, N], bf16, tag=f"xv{k}", name=f"xv{k}") for k in range(KT)]
        wq = [[cp.tile([128, 128], bf16, tag=f"wq{p}{k}", name=f"wq{p}{k}")
               for k in range(KT)] for p in range(2)]
        wk = [[cp.tile([128, 128], bf16, tag=f"wk{p}{k}", name=f"wk{p}{k}")
               for k in range(KT)] for p in range(2)]
        wv = [cp.tile([128, HPC * E], bf16, tag=f"wv{k}", name=f"wv{k}")
              for k in range(KT)]
        wo = [cp.tile([E, D], bf16, tag=f"wo{h}", name=f"wo{h}") for h in range(HPC)]
        qt = [cp.tile([128, N], bf16, tag=f"qt{p}", name=f"qt{p}") for p in range(2)]
        kt = [cp.tile([128, N], bf16, tag=f"kt{p}", name=f"kt{p}") for p in range(2)]
        vaug = [cp.tile([128, HPC, VSLOT], bf16, tag=f"va{t}", name=f"va{t}")
                for t in range(NKT)]
        repbf16 = [cp.tile([E, N], bf16, tag=f"rb{h}", name=f"rb{h}")
                   for h in range(HPC)]
        onesb = cp.tile([65, E], bf16, tag="onesb")

        # --- input DMAs, deadline-ordered. scalar queue: only head loads
        # (finish before the first exp); sync queue: everything else.
        nc.gpsimd.memset(onesb[:], 1.0)
        c0 = slice(0, 512)
        for k in range(KT):
            nc.sync.dma_start(wk[0][k][:], wkT[0, k])
            nc.scalar.dma_start(wq[0][k][:], wqT[0, k])
        for k in range(KT):
            nc.sync.dma_start(xk[k][:, c0], xkT[k][:, c0])
            nc.scalar.dma_start(xq[k][:, c0], xqT[k][:, c0])
        for k in range(KT):
            nc.scalar.dma_start(wv[k][:], wvT[k])
        for c in range(4):          # xv c0, xk c1, xv c1, xk c2, ...
            sl = slice(c * 512, (c + 1) * 512)
            for k in range(KT):
                nc.sync.dma_start(xv[k][:, sl], vT[k][:, sl])
            if c < 3:
                sl2 = slice((c + 1) * 512, (c + 2) * 512)
                for k in range(KT):
                    nc.sync.dma_start(xk[k][:, sl2], xkT[k][:, sl2])
        sl = slice(512, 1024)
        for k in range(KT):
            nc.sync.dma_start(xq[k][:, sl], xqT[k][:, sl])
        for k in range(KT):
            nc.sync.dma_start(wk[1][k][:], wkT[1, k])
            nc.sync.dma_start(wq[1][k][:], wqT[1, k])
        for c in range(2, 4):
            sl = slice(c * 512, (c + 1) * 512)
            for k in range(KT):
                nc.sync.dma_start(xq[k][:, sl], xqT[k][:, sl])
        for h in range(HPC):
            nc.sync.dma_start(wo[h][:], woT[h])

        # --- PE warmup burst: dependency-free dummy matmuls fill the DMA
        # window and push HAM to K=8/8 before the first projection.
        warm_sb = cp.tile([128, 512], bf16, tag="warm_sb")
        nc.gpsimd.memset(warm_sb[:], 0.0)
        with tc.tile_pool(name="warmps", bufs=1, space="PSUM") as wps:
            wpt = wps.tile([128, 512], f32, tag="w", name="warm_ps")
            for i in range(12):
                nc.tensor.matmul(wpt[:], warm_sb[:, 0:128], warm_sb[:],
                                 start=True, stop=True)

        # --- PSUM pools: spair 2 banks x 2 bufs + rep 2 x 1 bank + fill
        # 2 x 1 bank = 8 banks.
        sp = ctx.enter_context(tc.tile_pool(name="spsum", bufs=2, space="PSUM"))
        rp = ctx.enter_context(tc.tile_pool(name="rpsum", bufs=1, space="PSUM"))
        fpp = ctx.enter_context(tc.tile_pool(name="fill", bufs=2, space="PSUM"))
        ptp = ctx.enter_context(tc.tile_pool(name="ptile", bufs=4))
        smp = ctx.enter_context(tc.tile_pool(name="small", bufs=2))

        def proj_chunk(dst, w, x, c):
            ps = fpp.tile([128, 512], f32, tag="f", name="proj_ps")
            sl = slice(c * 512, (c + 1) * 512)
            for k in range(KT):
                nc.tensor.matmul(
                    ps[:], w[k][:], x[k][:, sl],
                    start=(k == 0), stop=(k == KT - 1),
                )
            nc.vector.tensor_copy(dst[:, sl], ps[:])

        def vproj_tile(t):
            nc.gpsimd.memset(vaug[t][:], 1.0)
            ps = fpp.tile([128, HPC * E], f32, tag="f", name="vproj_ps")
            tsl = slice(t * 128, (t + 1) * 128)
            for k in range(KT):
                nc.tensor.matmul(
                    ps[:], xv[k][:, tsl], wv[k][:],
                    start=(k == 0), stop=(k == KT - 1),
                )
            nc.vector.tensor_copy(vaug[t][:, :, 0:E], ps[:])

        # --- minimal head: just enough projection for attention (p0, q0).
        proj_chunk(kt[0], wk[0], xk, 0)
        proj_chunk(qt[0], wq[0], xq, 0)

        def attention_quarter(p, q4, pre=None, post=None):
            # Emission is software-pipelined one tile ahead: PE queue order
            # is S(0), S(1), [pre0], PV(0), S(2), [pre1], PV(1), ... so a
            # stalled pre-hook/PV never delays the next S (and with it the
            # exp stream). pre[t] runs before PV(t) (e.g. vproj(t)).
            pre = pre or {}
            post = post or {}
            qoff = q4 * QW
            rep_ps = [
                rp.tile([65, QW], f32, tag=f"rep{s}", name=f"rep{s}")
                for s in range(2)
            ]
            spair = [None, None]
            pt = [None, None]

            def emit_s(t):
                tsl = slice(t * 128, (t + 1) * 128)
                sb = sp.tile([128, 2 * QW], f32, tag="s", name="spair")
                spair[t % 2] = sb
                for s in range(2):
                    esl = slice(s * 64, (s + 1) * 64)
                    nc.tensor.matmul(
                        sb[:, s * QW:(s + 1) * QW],
                        kt[p][esl, tsl], qt[p][esl, qoff:qoff + QW],
                        start=True, stop=True,
                    )

            emit_s(0)
            for t in range(NKT):
                if t + 1 < NKT:
                    emit_s(t + 1)
                ptt = ptp.tile([128, 2 * QW], bf16, tag="p", name="pt")
                nc.scalar.activation(ptt[:], spair[t % 2][:], Exp)
                for fn in pre.get(t, ()):
                    fn()
                for s in range(2):
                    h = 2 * p + s
                    nc.tensor.matmul(
                        rep_ps[s][:],
                        vaug[t][:, h, 0:65], ptt[:, s * QW:(s + 1) * QW],
                        start=(t == 0), stop=(t == NKT - 1),
                    )
                for fn in post.get(t, ()):
                    fn()
            # drain rep PSUM immediately (frees the banks for the next
            # quarter); the normalize chain is deferred.
            rrawb = []
            for s in range(2):
                rr = smp.tile([65, QW], bf16, tag=f"rr{s}", name=f"rr{s}")
                nc.vector.tensor_copy(rr[:], rep_ps[s][:])
                rrawb.append(rr)
            return rrawb

        def norm_chain(p, q4, rrawb):
            # d (row 64 of rep) -> K=1 broadcast matmul over 64 partitions
            # -> fast reciprocal -> scale rep into repbf16. Partition-aligned.
            qoff = q4 * QW
            for s in range(2):
                h = 2 * p + s
                dbp = fpp.tile([E, QW], f32, tag="f", name="dbp")
                nc.tensor.matmul(dbp[:], onesb[64:65, :], rrawb[s][64:65, :],
                                 start=True, stop=True)
                dinvb = smp.tile([E, QW], f32, tag=f"dv{s}", name="dinvb")
                with nc.allow_low_precision(reason="softmax denom reciprocal"):
                    nc.vector.reciprocal_approx_fast(dinvb[:], dbp[:])
                    nc.vector.tensor_mul(
                        repbf16[h][:, qoff:qoff + QW], rrawb[s][0:E, :], dinvb[:]
                    )

        def outproj_tile(t):
            tsl = slice(t * 128, (t + 1) * 128)
            ops = fpp.tile([128, D], f32, tag="f", name="ops")
            for h in range(HPC):
                nc.tensor.matmul(
                    ops[:], repbf16[h][:, tsl], wo[h][:],
                    start=(h == 0), stop=(h == HPC - 1),
                )
            ost = ptp.tile([128, D], bf16, tag="ost")
            nc.vector.tensor_copy(ost[:], ops[:])
            nc.sync.dma_start(outp[t], ost[:])

        chains = {}

        def run_chain(p, q4):
            return lambda: norm_chain(p, q4, chains.pop((p, q4)))

        # --- pair 0 (projections for later quarters/pair ride as post
        # hooks in PE gaps; every producer is emitted before its consumer).
        chains[(0, 0)] = attention_quarter(
            0, 0,
            pre={t: [lambda t=t: vproj_tile(t)] for t in range(NKT)},
            post={
                2: [lambda: proj_chunk(kt[0], wk[0], xk, 1)],
                6: [lambda: proj_chunk(kt[0], wk[0], xk, 2)],
                10: [lambda: proj_chunk(kt[0], wk[0], xk, 3)],
                15: [lambda: proj_chunk(qt[0], wq[0], xq, 1)],
            })
        chains[(0, 1)] = attention_quarter(
            0, 1,
            post={
                1: [run_chain(0, 0)],
                4: [lambda: proj_chunk(kt[1], wk[1], xk, 0)],
                7: [lambda: proj_chunk(kt[1], wk[1], xk, 1)],
                10: [lambda: proj_chunk(kt[1], wk[1], xk, 2)],
                13: [lambda: proj_chunk(qt[0], wq[0], xq, 2)],
            })
        chains[(0, 2)] = attention_quarter(
            0, 2,
            post={
                1: [run_chain(0, 1)],
                4: [lambda: proj_chunk(kt[1], wk[1], xk, 3)],
                7: [lambda: proj_chunk(qt[1], wq[1], xq, 0)],
                10: [lambda: proj_chunk(qt[1], wq[1], xq, 1)],
                13: [lambda: proj_chunk(qt[0], wq[0], xq, 3)],
            })
        chains[(0, 3)] = attention_quarter(
            0, 3,
            post={
                1: [run_chain(0, 2)],
                5: [lambda: proj_chunk(qt[1], wq[1], xq, 2)],
                9: [lambda: proj_chunk(qt[1], wq[1], xq, 3)],
            })
        # --- pair 1 (output projection per nq tile, one quarter behind).
        chains[(1, 0)] = attention_quarter(
            1, 0, post={1: [run_chain(0, 3)]})
        chains[(1, 1)] = attention_quarter(
            1, 1,
            post={1: [run_chain(1, 0)],
                  4: [lambda: outproj_tile(0)], 7: [lambda: outproj_tile(1)],
                  10: [lambda: outproj_tile(2)], 13: [lambda: outproj_tile(3)]})
        chains[(1, 2)] = attention_quarter(
            1, 2,
            post={1: [run_chain(1, 1)],
                  4: [lambda: outproj_tile(4)], 7: [lambda: outproj_tile(5)],
                  10: [lambda: outproj_tile(6)], 13: [lambda: outproj_tile(7)]})
        chains[(1, 3)] = attention_quarter(
            1, 3,
            post={1: [run_chain(1, 2)],
                  4: [lambda: outproj_tile(8)], 7: [lambda: outproj_tile(9)],
                  10: [lambda: outproj_tile(10)], 13: [lambda: outproj_tile(11)]})
        run_chain(1, 3)()
        for t in range(12, 16):
            outproj_tile(t)

    nc.compile()
    return nc


def _prep_core_inputs(c, x1, x2, v, Wq, Wk, Wv, Wo, identf=None, identb=None):
    bf = ml_dtypes.bfloat16
    b, g = c // 2, c % 2
    hs = slice(g * HPC, (g + 1) * HPC)
    wq = (Wq[hs] * (1.0 / np.sqrt(E))).astype(np.float32)   # fold 1/sqrt(E)
    wk, wv, wo = Wk[hs], Wv[hs], Wo[hs]

    def t_pack_pair(w):
        # [4,E,D] -> per pair p: concat(w[2p].T, w[2p+1].T, axis=1) [D,128]
        out = np.empty((2, KT, 128, 128), bf)
        for p in range(2):
            m = np.concatenate([w[2 * p].T, w[2 * p + 1].T], axis=1)  # [D,128]
            out[p] = m.reshape(KT, 128, 128).astype(bf)
        return out

    xq = np.ascontiguousarray(x2[b].T).astype(bf).reshape(KT, 128, N)
    xk = np.ascontiguousarray(x1[b].T).astype(bf).reshape(KT, 128, N)
    xv = np.ascontiguousarray(v[b].T).astype(bf).reshape(KT, 128, N)
    wvT = np.concatenate([wv[h].T for h in range(HPC)], axis=1)  # [D, 256]
    woT = np.stack([wo[h].T for h in range(HPC)])                # [4, E, D]
    return {
        "xqT": xq, "xkT": xk, "vT": xv,
        "wqT": t_pack_pair(wq), "wkT": t_pack_pair(wk),
        "wvT": np.ascontiguousarray(wvT).astype(bf).reshape(KT, 128, HPC * E),
        "woT": woT.astype(bf),
    }


def kernel(**inputs):
    from concourse.bass_utils import run_bass_kernel_spmd

    x1 = np.asarray(inputs["x1"], np.float32)
    x2 = np.asarray(inputs["x2"], np.float32)
    v = np.asarray(inputs["v"], np.float32)
    Wq = np.asarray(inputs["Wq"], np.float32)
    Wk = np.asarray(inputs["Wk"], np.float32)
    Wv = np.asarray(inputs["Wv"], np.float32)
    Wo = np.asarray(inputs["Wo"], np.float32)

    if "nc" not in _CACHE:
        _CACHE["nc"] = _build()
    nc = _CACHE["nc"]

    in_maps = [
        _prep_core_inputs(c, x1, x2, v, Wq, Wk, Wv, Wo)
        for c in range(N_CORES)
    ]
    res = run_bass_kernel_spmd(nc, in_maps, list(range(N_CORES)))
    out = np.empty((B, N, D), np.float32)
    for b in range(B):
        out[b] = (
            res.results[2 * b]["outp"].reshape(N, D).astype(np.float32)
            + res.results[2 * b + 1]["outp"].reshape(N, D).astype(np.float32)
        )
    return out


# revision 8
# speedup vs baseline: 1.3612x; 1.0554x over previous
"""Trainium2 Bass kernel for nn_Attention_91293824844283.

Multi-head attention (identity rep): per-head 1x1-conv Q/K/V projections,
softmax(Q K^T / sqrt(E)) V, per-head output projection summed over heads.

Shapes: B=4, N=2048, D=512, H=8, E=64.

Sharding over 8 cores: core c -> (batch b = c//2, head-group g = c%2 of 4
heads). Each core computes the partial output sum over its 4 heads for its
batch; host adds the two partials per batch.

Device-side pipeline (per core), ScalarE(exp)-paced at ~1.34us per
[128,1024] tile (128 tiles of exp is the hard floor; all PE work hides
under it):
  - Flat 128-step stream over (pair, quarter, nk-tile). Step i emits
    S(i) + exp(i); PV runs SKEW=8 steps behind, buffered in SBUF pt
    tiles. The skew absorbs the projection-heavy start (all of V and K
    projections are structurally pinned inside the first quarter) and
    decouples PV stalls from the S->exp critical chain.
  - S^T per head-pair: two K=64 matmuls in disjoint PE row groups share
    one [128,1024] PSUM tile; one ACT exp per tile. V [N, 4 slots of 66]
    with a ones column per head so PV also produces the softmax
    denominators (M=65).
  - Normalization without PE transposes: rep PSUM is drained to SBUF
    immediately (frees the banks); a deferred chain does d-row K=1
    broadcast matmul -> reciprocal_approx_fast -> scale. Head s=1 of
    each pair is scaled on GPSIMD writing partitions 64:128, building a
    pair-stacked rep so the output projection contracts K=128.
  - Output projection: 2 K=128 matmuls accumulate out[nq,D] in PSUM,
    cast bf16, DMA. Host sums the two half-head partials in f32.
  - DMA: sync + scalar HWDGE queues for head-critical loads (scalar goes
    exp-only after ~13us), xv/wv on the GPSIMD SWDGE queue in parallel.
"""

import numpy as np
import ml_dtypes
from contextlib import ExitStack

B, N, D, H, E = 4, 2048, 512, 8, 64
HPC = 4            # heads per core
N_CORES = 8
NKT = N // 128     # 16 nk tiles
VSLOT = 66         # V slot: 64 V cols + 1 ones col + 1 pad
KT = D // 128      # 4 contraction tiles for projections
QW = 512           # nq quarter width
SKEW = 8           # PV lag (in tiles) behind the S/exp stream

_CACHE = {}


def _build():
    import concourse.tile as tile
    from concourse import bacc, mybir

    bf16 = mybir.dt.bfloat16
    f32 = mybir.dt.float32
    Exp = mybir.ActivationFunctionType.Exp

    nc = bacc.Bacc(
        "TRN2", target_bir_lowering=False, debug=False, num_devices=N_CORES
    )
    xqT = nc.dram_tensor("xqT", [KT, 128, N], bf16, kind="ExternalInput").ap()
    xkT = nc.dram_tensor("xkT", [KT, 128, N], bf16, kind="ExternalInput").ap()
    vT = nc.dram_tensor("vT", [KT, 128, N], bf16, kind="ExternalInput").ap()
    wqT = nc.dram_tensor("wqT", [2, KT, 128, 128], bf16, kind="ExternalInput").ap()
    wkT = nc.dram_tensor("wkT", [2, KT, 128, 128], bf16, kind="ExternalInput").ap()
    wvT = nc.dram_tensor("wvT", [KT, 128, HPC * E], bf16, kind="ExternalInput").ap()
    woT = nc.dram_tensor("woT", [2, 128, D], bf16, kind="ExternalInput").ap()
    outp = nc.dram_tensor("outp", [NKT, 128, D], bf16, kind="ExternalOutput").ap()

    with tile.TileContext(nc) as tc, ExitStack() as ctx:
        cp = ctx.enter_context(tc.tile_pool(name="const", bufs=1))

        # --- persistent SBUF tiles ---
        xq = [cp.tile([128, N], bf16, tag=f"xq{k}", name=f"xq{k}") for k in range(KT)]
        xk = [cp.tile([128, N], bf16, tag=f"xk{k}", name=f"xk{k}") for k in range(KT)]
        xv = [cp.tile([128, N], bf16, tag=f"xv{k}", name=f"xv{k}") for k in range(KT)]
        wq = [[cp.tile([128, 128], bf16, tag=f"wq{p}{k}", name=f"wq{p}{k}")
               for k in range(KT)] for p in range(2)]
        wk = [[cp.tile([128, 128], bf16, tag=f"wk{p}{k}", name=f"wk{p}{k}")
               for k in range(KT)] for p in range(2)]
        wv = [cp.tile([128, HPC * E], bf16, tag=f"wv{k}", name=f"wv{k}")
              for k in range(KT)]
        wost = [cp.tile([128, D], bf16, tag=f"wo{p}", name=f"wo{p}")
                for p in range(2)]
        qt = [cp.tile([128, N], bf16, tag=f"qt{p}", name=f"qt{p}") for p in range(2)]
        kt = [cp.tile([128, N], bf16, tag=f"kt{p}", name=f"kt{p}") for p in range(2)]
        vaug = [cp.tile([128, HPC, VSLOT], bf16, tag=f"va{t}", name=f"va{t}")
                for t in range(NKT)]
        repst = [cp.tile([128, N], bf16, tag=f"rs{p}", name=f"rs{p}")
                 for p in range(2)]
        onesb = cp.tile([65, E], bf16, tag="onesb")

        # --- input DMAs, deadline-ordered across three queues.
        nc.gpsimd.memset(onesb[:], 1.0)
        c0 = slice(0, 512)
        for k in range(KT):          # head-critical, k-interleaved
            nc.sync.dma_start(wk[0][k][:], wkT[0, k])
            nc.sync.dma_start(xk[k][:, c0], xkT[k][:, c0])
            nc.scalar.dma_start(wq[0][k][:], wqT[0, k])
            nc.scalar.dma_start(xq[k][:, c0], xqT[k][:, c0])
        for k in range(KT):          # V path on the SWDGE queue
            nc.gpsimd.dma_start(wv[k][:], wvT[k])
        for c in range(4):
            sl = slice(c * 512, (c + 1) * 512)
            for k in range(KT):
                nc.gpsimd.dma_start(xv[k][:, sl], vT[k][:, sl])
        for c in (1, 2):
            sl = slice(c * 512, (c + 1) * 512)
            for k in range(KT):
                nc.sync.dma_start(xk[k][:, sl], xkT[k][:, sl])
        sl = slice(512, 1024)
        for k in range(KT):
            nc.sync.dma_start(xq[k][:, sl], xqT[k][:, sl])
        sl = slice(3 * 512, 4 * 512)
        for k in range(KT):
            nc.sync.dma_start(xk[k][:, sl], xkT[k][:, sl])
        for k in range(KT):
            nc.sync.dma_start(wk[1][k][:], wkT[1, k])
            nc.sync.dma_start(wq[1][k][:], wqT[1, k])
        for c in (2, 3):
            sl = slice(c * 512, (c + 1) * 512)
            for k in range(KT):
                nc.sync.dma_start(xq[k][:, sl], xqT[k][:, sl])
        for p in range(2):
            nc.sync.dma_start(wost[p][:], woT[p])

        # --- PE warmup burst: dependency-free dummy matmuls fill the DMA
        # window and push HAM to K=8/8 before the first projection.
        warm_sb = cp.tile([128, 512], bf16, tag="warm_sb")
        nc.gpsimd.memset(warm_sb[:], 0.0)
        with tc.tile_pool(name="warmps", bufs=1, space="PSUM") as wps:
            wpt = wps.tile([128, 512], f32, tag="w", name="warm_ps")
            for i in range(12):
                nc.tensor.matmul(wpt[:], warm_sb[:, 0:128], warm_sb[:],
                                 start=True, stop=True)

        # --- PSUM pools: spair 2 banks x 2 bufs + rep 2 x 1 bank + fill
        # 2 x 1 bank = 8 banks.
        sp = ctx.enter_context(tc.tile_pool(name="spsum", bufs=2, space="PSUM"))
        rp = ctx.enter_context(tc.tile_pool(name="rpsum", bufs=1, space="PSUM"))
        fpp = ctx.enter_context(tc.tile_pool(name="fill", bufs=2, space="PSUM"))
        ptp = ctx.enter_context(tc.tile_pool(name="ptile", bufs=SKEW + 3))
        smp = ctx.enter_context(tc.tile_pool(name="small", bufs=2))

        def proj_chunk(dst, w, x, c):
            ps = fpp.tile([128, 512], f32, tag="f", name="proj_ps")
            sl = slice(c * 512, (c + 1) * 512)
            for k in range(KT):
                nc.tensor.matmul(
                    ps[:], w[k][:], x[k][:, sl],
                    start=(k == 0), stop=(k == KT - 1),
                )
            nc.vector.tensor_copy(dst[:, sl], ps[:])

        def vproj_tile(t):
            nc.gpsimd.memset(vaug[t][:], 1.0)
            ps = fpp.tile([128, HPC * E], f32, tag="f", name="vproj_ps")
            tsl = slice(t * 128, (t + 1) * 128)
            for k in range(KT):
                nc.tensor.matmul(
                    ps[:], xv[k][:, tsl], wv[k][:],
                    start=(k == 0), stop=(k == KT - 1),
                )
            nc.vector.tensor_copy(vaug[t][:, :, 0:E], ps[:])

        # --- minimal head: just enough projection for the stream start.
        proj_chunk(kt[0], wk[0], xk, 0)
        proj_chunk(qt[0], wq[0], xq, 0)

        def norm_chain(k8, rrawb):
            # d (row 64 of rep~) -> K=1 broadcast matmul over 64 partitions
            # -> fast reciprocal -> scale into the pair-stacked rep. s=0 on
            # DVE (partitions 0:64), s=1 on GPSIMD writing 64:128.
            p, q4 = k8 // 4, k8 % 4
            qsl = slice(q4 * QW, (q4 + 1) * QW)
            for s in range(2):
                dbp = fpp.tile([E, QW], f32, tag="f", name="dbp")
                nc.tensor.matmul(dbp[:], onesb[64:65, :], rrawb[s][64:65, :],
                                 start=True, stop=True)
                dinvb = smp.tile([E, QW], f32, tag=f"dv{s}", name="dinvb")
                with nc.allow_low_precision(reason="softmax denom reciprocal"):
                    nc.vector.reciprocal_approx_fast(dinvb[:], dbp[:])
                    if s == 0:
                        nc.vector.tensor_mul(
                            repst[p][0:E, qsl], rrawb[s][0:E, :], dinvb[:])
                    else:
                        nc.gpsimd.tensor_mul(
                            repst[p][E:128, qsl], rrawb[s][0:E, :], dinvb[:])

        def outproj_tile(t):
            tsl = slice(t * 128, (t + 1) * 128)
            ops = fpp.tile([128, D], f32, tag="f", name="ops")
            for p in range(2):
                nc.tensor.matmul(
                    ops[:], repst[p][:, tsl], wost[p][:],
                    start=(p == 0), stop=(p == 1),
                )
            ost = ptp.tile([128, D], bf16, tag="ost")
            nc.vector.tensor_copy(ost[:], ops[:])
            nc.sync.dma_start(outp[t], ost[:])

        # --- flat skewed stream ---------------------------------------
        NSTEP = 2 * 4 * NKT          # 128
        rep_ps = {}                  # quarter-index -> [rep_ps tiles]
        rrawb = {}                   # quarter-index -> [sbuf drains]
        pts = {}                     # step -> pt tile

        hooks = {}

        def add_hook(i, fn):
            hooks.setdefault(i, []).append(fn)

        # projections (consumer step -> emit 2 steps early)
        add_hook(2, lambda: proj_chunk(kt[0], wk[0], xk, 1))
        add_hook(6, lambda: proj_chunk(kt[0], wk[0], xk, 2))
        add_hook(10, lambda: proj_chunk(kt[0], wk[0], xk, 3))
        add_hook(14, lambda: proj_chunk(qt[0], wq[0], xq, 1))
        add_hook(30, lambda: proj_chunk(qt[0], wq[0], xq, 2))
        add_hook(46, lambda: proj_chunk(qt[0], wq[0], xq, 3))
        add_hook(20, lambda: proj_chunk(kt[1], wk[1], xk, 0))
        add_hook(24, lambda: proj_chunk(kt[1], wk[1], xk, 1))
        add_hook(28, lambda: proj_chunk(kt[1], wk[1], xk, 2))
        add_hook(32, lambda: proj_chunk(kt[1], wk[1], xk, 3))
        add_hook(36, lambda: proj_chunk(qt[1], wq[1], xq, 0))
        add_hook(40, lambda: proj_chunk(qt[1], wq[1], xq, 1))
        add_hook(44, lambda: proj_chunk(qt[1], wq[1], xq, 2))
        add_hook(50, lambda: proj_chunk(qt[1], wq[1], xq, 3))
        # vproj(t) just before PV(0,0,t) at step t+SKEW
        for t in range(NKT):
            add_hook(max(0, t + SKEW - 1), lambda t=t: vproj_tile(t))
        # normalize chain for quarter k8 (PV done at step 16*k8+15+SKEW)
        for k8 in range(7):
            add_hook(16 * k8 + 15 + SKEW + 2,
                     lambda k8=k8: norm_chain(k8, rrawb.pop(k8)))
        # output projection: quarter k8 (pair1 q = k8-4) tiles 4q..4q+3
        for k8 in range(4, 7):
            for j in range(4):
                add_hook(16 * k8 + 15 + SKEW + 4 + 2 * j,
                         lambda k8=k8, j=j: outproj_tile(4 * (k8 - 4) + j))

        spair = {}
        for i in range(NSTEP + SKEW):
            if i < NSTEP:
                p, q4, t = i // 64, (i // 16) % 4, i % 16
                k8 = i // 16
                if t == 0:
                    rep_ps[k8] = [
                        rp.tile([65, QW], f32, tag=f"rep{s}", name=f"rep{s}")
                        for s in range(2)
                    ]
                tsl = slice(t * 128, (t + 1) * 128)
                qoff = q4 * QW
                sb = sp.tile([128, 2 * QW], f32, tag="s", name="spair")
                spair[i] = sb
                for s in range(2):
                    esl = slice(s * 64, (s + 1) * 64)
                    nc.tensor.matmul(
                        sb[:, s * QW:(s + 1) * QW],
                        kt[p][esl, tsl], qt[p][esl, qoff:qoff + QW],
                        start=True, stop=True,
                    )
                ptt = ptp.tile([128, 2 * QW], bf16, tag="p", name="pt")
                nc.scalar.activation(ptt[:], sb[:], Exp)
                pts[i] = ptt
                del spair[i]
            j = i - SKEW
            if 0 <= j < NSTEP:
                p, q4, t = j // 64, (j // 16) % 4, j % 16
                k8 = j // 16
                ptt = pts.pop(j)
                for s in range(2):
                    h = 2 * p + s
                    nc.tensor.matmul(
                        rep_ps[k8][s][:],
                        vaug[t][:, h, 0:65], ptt[:, s * QW:(s + 1) * QW],
                        start=(t == 0), stop=(t == NKT - 1),
                    )
                if t == NKT - 1:
                    rr = []
                    for s in range(2):
                        r = smp.tile([65, QW], bf16, tag=f"rr{s}", name=f"rr{s}")
                        nc.vector.tensor_copy(r[:], rep_ps[k8][s][:])
                        rr.append(r)
                    rrawb[k8] = rr
                    del rep_ps[k8]
            for fn in hooks.get(i, ()):
                fn()
        # --- tail: last quarter normalize + final output tiles
        norm_chain(7, rrawb.pop(7))
        for t in range(12, 16):
            outproj_tile(t)

    nc.compile()
    return nc


def _prep_core_inputs(c, x1, x2, v, Wq, Wk, Wv, Wo, identf=None, identb=None):
    bf = ml_dtypes.bfloat16
    b, g = c // 2, c % 2
    hs = slice(g * HPC, (g + 1) * HPC)
    wq = (Wq[hs] * (1.0 / np.sqrt(E))).astype(np.float32)   # fold 1/sqrt(E)
    wk, wv, wo = Wk[hs], Wv[hs], Wo[hs]

    def t_pack_pair(w):
        # [4,E,D] -> per pair p: concat(w[2p].T, w[2p+1].T, axis=1) [D,128]
        out = np.empty((2, KT, 128, 128), bf)
        for p in range(2):
            m = np.concatenate([w[2 * p].T, w[2 * p + 1].T], axis=1)  # [D,128]
            out[p] = m.reshape(KT, 128, 128).astype(bf)
        return out

    xq = np.ascontiguousarray(x2[b].T).astype(bf).reshape(KT, 128, N)
    xk = np.ascontiguousarray(x1[b].T).astype(bf).reshape(KT, 128, N)
    xv = np.ascontiguousarray(v[b].T).astype(bf).reshape(KT, 128, N)
    wvT = np.concatenate([wv[h].T for h in range(HPC)], axis=1)  # [D, 256]
    # pair-stacked output weights: [2, 128, D], rows = [E of h=2p; E of 2p+1]
    woT = np.stack([
        np.concatenate([wo[2 * p].T, wo[2 * p + 1].T], axis=0)
        for p in range(2)
    ])
    return {
        "xqT": xq, "xkT": xk, "vT": xv,
        "wqT": t_pack_pair(wq), "wkT": t_pack_pair(wk),
        "wvT": np.ascontiguousarray(wvT).astype(bf).reshape(KT, 128, HPC * E),
        "woT": woT.astype(bf),
    }


def kernel(**inputs):
    from concourse.bass_utils import run_bass_kernel_spmd

    x1 = np.asarray(inputs["x1"], np.float32)
    x2 = np.asarray(inputs["x2"], np.float32)
    v = np.asarray(inputs["v"], np.float32)
    Wq = np.asarray(inputs["Wq"], np.float32)
    Wk = np.asarray(inputs["Wk"], np.float32)
    Wv = np.asarray(inputs["Wv"], np.float32)
    Wo = np.asarray(inputs["Wo"], np.float32)

    if "nc" not in _CACHE:
        _CACHE["nc"] = _build()
    nc = _CACHE["nc"]

    in_maps = [
        _prep_core_inputs(c, x1, x2, v, Wq, Wk, Wv, Wo)
        for c in range(N_CORES)
    ]
    res = run_bass_kernel_spmd(nc, in_maps, list(range(N_CORES)))
    out = np.empty((B, N, D), np.float32)
    for b in range(B):
        out[b] = (
            res.results[2 * b]["outp"].reshape(N, D).astype(np.float32)
            + res.results[2 * b + 1]["outp"].reshape(N, D).astype(np.float32)
        )
    return out


# revision 11
# speedup vs baseline: 1.6358x; 1.2017x over previous
"""Trainium2 Bass kernel for nn_Attention_91293824844283.

Multi-head attention (identity rep): per-head 1x1-conv Q/K/V projections,
softmax(Q K^T / sqrt(E)) V, per-head output projection summed over heads.

Shapes: B=4, N=2048, D=512, H=8, E=64.

Sharding over 8 cores: core c -> (batch b = c//2, head-group g = c%2 of 4
heads). Each core computes the partial output sum over its 4 heads for its
batch; host adds the two partials per batch.

Device-side pipeline (per core), ScalarE(exp)-paced at ~1.34us per
[128,1024] tile (128 tiles of exp is the hard floor; all PE work hides
under it):
  - Flat 128-step stream over (pair, quarter, nk-tile). Step i emits
    S(i) + exp(i); PV runs SKEW=8 steps behind, buffered in SBUF pt
    tiles. The skew absorbs the projection-heavy start (all of V and K
    projections are structurally pinned inside the first quarter) and
    decouples PV stalls from the S->exp critical chain.
  - S^T per head-pair: two K=64 matmuls in disjoint PE row groups share
    one [128,1024] PSUM tile; one ACT exp per tile. V [N, 4 slots of 66]
    with a ones column per head so PV also produces the softmax
    denominators (M=65).
  - Normalization without PE transposes: rep PSUM is drained to SBUF
    immediately (frees the banks); a deferred chain does d-row K=1
    broadcast matmul -> reciprocal_approx_fast -> scale. Head s=1 of
    each pair is scaled on GPSIMD writing partitions 64:128, building a
    pair-stacked rep so the output projection contracts K=128.
  - Output projection: 2 K=128 matmuls accumulate out[nq,D] in PSUM,
    cast bf16, DMA. Host sums the two half-head partials in f32.
  - DMA: sync + scalar HWDGE queues for head-critical loads (scalar goes
    exp-only after ~13us), xv/wv on the GPSIMD SWDGE queue in parallel.
"""

import numpy as np
import ml_dtypes
from contextlib import ExitStack

B, N, D, H, E = 4, 2048, 512, 8, 64
HPC = 4            # heads per core
N_CORES = 8
NKT = N // 128     # 16 nk tiles
VSLOT = 66         # V slot: 64 V cols + 1 ones col + 1 pad
KT = D // 128      # 4 contraction tiles for projections
QW = 512           # nq quarter width
SKEW = 8           # PV lag (in tiles) behind the S/exp stream

_CACHE = {}


def _build():
    import concourse.tile as tile
    from concourse import bacc, mybir

    bf16 = mybir.dt.bfloat16
    f32 = mybir.dt.float32
    Exp = mybir.ActivationFunctionType.Exp

    nc = bacc.Bacc(
        "TRN2", target_bir_lowering=False, debug=False, num_devices=N_CORES
    )
    xqT = nc.dram_tensor("xqT", [KT, 128, N], bf16, kind="ExternalInput").ap()
    xkT = nc.dram_tensor("xkT", [KT, 128, N], bf16, kind="ExternalInput").ap()
    vT = nc.dram_tensor("vT", [KT, 128, N], bf16, kind="ExternalInput").ap()
    wqT = nc.dram_tensor("wqT", [2, KT, 128, 128], bf16, kind="ExternalInput").ap()
    wkT = nc.dram_tensor("wkT", [2, KT, 128, 128], bf16, kind="ExternalInput").ap()
    wvT = nc.dram_tensor("wvT", [KT, 128, HPC * E], bf16, kind="ExternalInput").ap()
    woT = nc.dram_tensor("woT", [2, 128, D], bf16, kind="ExternalInput").ap()
    outp = nc.dram_tensor("outp", [NKT, 128, D], bf16, kind="ExternalOutput").ap()

    with tile.TileContext(nc) as tc, ExitStack() as ctx:
        cp = ctx.enter_context(tc.tile_pool(name="const", bufs=1))

        # --- persistent SBUF tiles ---
        xq = [cp.tile([128, N], bf16, tag=f"xq{k}", name=f"xq{k}") for k in range(KT)]
        xk = [cp.tile([128, N], bf16, tag=f"xk{k}", name=f"xk{k}") for k in range(KT)]
        xv = [cp.tile([128, N], bf16, tag=f"xv{k}", name=f"xv{k}") for k in range(KT)]
        wq = [[cp.tile([128, 128], bf16, tag=f"wq{p}{k}", name=f"wq{p}{k}")
               for k in range(KT)] for p in range(2)]
        wk = [[cp.tile([128, 128], bf16, tag=f"wk{p}{k}", name=f"wk{p}{k}")
               for k in range(KT)] for p in range(2)]
        wv = [cp.tile([128, HPC * E], bf16, tag=f"wv{k}", name=f"wv{k}")
              for k in range(KT)]
        wost = [cp.tile([128, D], bf16, tag=f"wo{p}", name=f"wo{p}")
                for p in range(2)]
        qt = [cp.tile([128, N], bf16, tag=f"qt{p}", name=f"qt{p}") for p in range(2)]
        kt = [cp.tile([128, N], bf16, tag=f"kt{p}", name=f"kt{p}") for p in range(2)]
        vaug = [cp.tile([128, HPC, VSLOT], bf16, tag=f"va{t}", name=f"va{t}")
                for t in range(NKT)]
        repst = [cp.tile([128, N], bf16, tag=f"rs{p}", name=f"rs{p}")
                 for p in range(2)]
        onesb = cp.tile([65, E], bf16, tag="onesb")

        warm_sb = cp.tile([128, 512], bf16, tag="warm_sb")

        # --- input DMAs, deadline-ordered across three queues. All gpsimd
        # memsets ride ahead of / between the SWDGE DMAs so nothing on the
        # PE side ever waits behind a descriptor-gen burst.
        nc.gpsimd.memset(warm_sb[:], 0.0)
        nc.gpsimd.memset(onesb[:], 1.0)
        for t in range(4):
            nc.gpsimd.memset(vaug[t][:], 1.0)
        c0 = slice(0, 512)
        for k in range(KT):          # head-critical, k-interleaved
            nc.sync.dma_start(wk[0][k][:], wkT[0, k])
            nc.sync.dma_start(xk[k][:, c0], xkT[k][:, c0])
            nc.scalar.dma_start(wq[0][k][:], wqT[0, k])
            nc.scalar.dma_start(xq[k][:, c0], xqT[k][:, c0])
        for k in range(KT):          # V path on the SWDGE queue
            nc.gpsimd.dma_start(wv[k][:], wvT[k])
        for c in range(4):
            sl = slice(c * 512, (c + 1) * 512)
            for k in range(KT):
                nc.gpsimd.dma_start(xv[k][:, sl], vT[k][:, sl])
            if c < 3:
                for t in range(4 * (c + 1), 4 * (c + 2)):
                    nc.gpsimd.memset(vaug[t][:], 1.0)
        for c in (1, 2):
            sl = slice(c * 512, (c + 1) * 512)
            for k in range(KT):
                nc.sync.dma_start(xk[k][:, sl], xkT[k][:, sl])
        sl = slice(512, 1024)
        for k in range(KT):
            nc.sync.dma_start(xq[k][:, sl], xqT[k][:, sl])
        sl = slice(3 * 512, 4 * 512)
        for k in range(KT):
            nc.sync.dma_start(xk[k][:, sl], xkT[k][:, sl])
        for k in range(KT):
            nc.sync.dma_start(wk[1][k][:], wkT[1, k])
            nc.sync.dma_start(wq[1][k][:], wqT[1, k])
        for c in (2, 3):
            sl = slice(c * 512, (c + 1) * 512)
            for k in range(KT):
                nc.sync.dma_start(xq[k][:, sl], xqT[k][:, sl])
        for p in range(2):
            nc.sync.dma_start(wost[p][:], woT[p])

        # --- PE warmup burst: dependency-free dummy matmuls fill the DMA
        # window and push HAM to K=8/8 before the first projection.
        with tc.tile_pool(name="warmps", bufs=1, space="PSUM") as wps:
            wpt = wps.tile([128, 512], f32, tag="w", name="warm_ps")
            for i in range(12):
                nc.tensor.matmul(wpt[:], warm_sb[:, 0:128], warm_sb[:],
                                 start=True, stop=True)

        # --- PSUM pools: spair 2 banks x 2 bufs + rep 2 x 1 bank + fill
        # 2 x 1 bank = 8 banks.
        sp = ctx.enter_context(tc.tile_pool(name="spsum", bufs=2, space="PSUM"))
        rp = ctx.enter_context(tc.tile_pool(name="rpsum", bufs=1, space="PSUM"))
        fpp = ctx.enter_context(tc.tile_pool(name="fill", bufs=2, space="PSUM"))
        ptp = ctx.enter_context(tc.tile_pool(name="ptile", bufs=SKEW + 3))
        smp = ctx.enter_context(tc.tile_pool(name="small", bufs=2))

        def proj_chunk(dst, w, x, c):
            ps = fpp.tile([128, 512], f32, tag="f", name="proj_ps")
            sl = slice(c * 512, (c + 1) * 512)
            for k in range(KT):
                nc.tensor.matmul(
                    ps[:], w[k][:], x[k][:, sl],
                    start=(k == 0), stop=(k == KT - 1),
                )
            nc.vector.tensor_copy(dst[:, sl], ps[:])

        def vproj_tile(t):
            ps = fpp.tile([128, HPC * E], f32, tag="f", name="vproj_ps")
            tsl = slice(t * 128, (t + 1) * 128)
            for k in range(KT):
                nc.tensor.matmul(
                    ps[:], xv[k][:, tsl], wv[k][:],
                    start=(k == 0), stop=(k == KT - 1),
                )
            nc.vector.tensor_copy(vaug[t][:, :, 0:E], ps[:])

        # --- minimal head: just enough projection for the stream start.
        proj_chunk(kt[0], wk[0], xk, 0)
        proj_chunk(qt[0], wq[0], xq, 0)

        def norm_chain(k8, rrawb):
            # d (row 64 of rep~) -> K=1 broadcast matmul over 64 partitions
            # -> fast reciprocal -> scale into the pair-stacked rep. s=0 on
            # DVE (partitions 0:64), s=1 on GPSIMD writing 64:128.
            p, q4 = k8 // 4, k8 % 4
            qsl = slice(q4 * QW, (q4 + 1) * QW)
            for s in range(2):
                dbp = fpp.tile([E, QW], f32, tag="f", name="dbp")
                nc.tensor.matmul(dbp[:], onesb[64:65, :], rrawb[s][64:65, :],
                                 start=True, stop=True)
                dinvb = smp.tile([E, QW], f32, tag=f"dv{s}", name="dinvb")
                with nc.allow_low_precision(reason="softmax denom reciprocal"):
                    nc.vector.reciprocal_approx_fast(dinvb[:], dbp[:])
                    if s == 0:
                        nc.vector.tensor_mul(
                            repst[p][0:E, qsl], rrawb[s][0:E, :], dinvb[:])
                    else:
                        nc.gpsimd.tensor_mul(
                            repst[p][E:128, qsl], rrawb[s][0:E, :], dinvb[:])

        def outproj_tile(t):
            tsl = slice(t * 128, (t + 1) * 128)
            ops = fpp.tile([128, D], f32, tag="f", name="ops")
            for p in range(2):
                nc.tensor.matmul(
                    ops[:], repst[p][:, tsl], wost[p][:],
                    start=(p == 0), stop=(p == 1),
                )
            ost = ptp.tile([128, D], bf16, tag="ost")
            nc.vector.tensor_copy(ost[:], ops[:])
            nc.sync.dma_start(outp[t], ost[:])

        # --- flat skewed stream ---------------------------------------
        NSTEP = 2 * 4 * NKT          # 128
        rep_ps = {}                  # quarter-index -> [rep_ps tiles]
        rrawb = {}                   # quarter-index -> [sbuf drains]
        pts = {}                     # step -> pt tile

        hooks = {}

        def add_hook(i, fn):
            hooks.setdefault(i, []).append(fn)

        # projections (consumer step -> emit 2 steps early)
        add_hook(2, lambda: proj_chunk(kt[0], wk[0], xk, 1))
        add_hook(6, lambda: proj_chunk(kt[0], wk[0], xk, 2))
        add_hook(10, lambda: proj_chunk(kt[0], wk[0], xk, 3))
        add_hook(14, lambda: proj_chunk(qt[0], wq[0], xq, 1))
        add_hook(30, lambda: proj_chunk(qt[0], wq[0], xq, 2))
        add_hook(46, lambda: proj_chunk(qt[0], wq[0], xq, 3))
        add_hook(20, lambda: proj_chunk(kt[1], wk[1], xk, 0))
        add_hook(24, lambda: proj_chunk(kt[1], wk[1], xk, 1))
        add_hook(28, lambda: proj_chunk(kt[1], wk[1], xk, 2))
        add_hook(32, lambda: proj_chunk(kt[1], wk[1], xk, 3))
        add_hook(36, lambda: proj_chunk(qt[1], wq[1], xq, 0))
        add_hook(40, lambda: proj_chunk(qt[1], wq[1], xq, 1))
        add_hook(44, lambda: proj_chunk(qt[1], wq[1], xq, 2))
        add_hook(50, lambda: proj_chunk(qt[1], wq[1], xq, 3))
        # vproj(t) just before PV(0,0,t) at step t+SKEW
        for t in range(NKT):
            add_hook(max(0, t + SKEW - 1), lambda t=t: vproj_tile(t))
        # normalize chain for quarter k8 (PV done at step 16*k8+15+SKEW)
        for k8 in range(7):
            add_hook(16 * k8 + 15 + SKEW + 2,
                     lambda k8=k8: norm_chain(k8, rrawb.pop(k8)))
        # output projection: quarter k8 (pair1 q = k8-4) tiles 4q..4q+3
        for k8 in range(4, 7):
            for j in range(4):
                add_hook(16 * k8 + 15 + SKEW + 4 + 2 * j,
                         lambda k8=k8, j=j: outproj_tile(4 * (k8 - 4) + j))

        spair = {}
        for i in range(NSTEP + SKEW):
            if i < NSTEP:
                p, q4, t = i // 64, (i // 16) % 4, i % 16
                k8 = i // 16
                if t == 0:
                    rep_ps[k8] = [
                        rp.tile([65, QW], f32, tag=f"rep{s}", name=f"rep{s}")
                        for s in range(2)
                    ]
                tsl = slice(t * 128, (t + 1) * 128)
                qoff = q4 * QW
                sb = sp.tile([128, 2 * QW], f32, tag="s", name="spair")
                spair[i] = sb
                for s in range(2):
                    esl = slice(s * 64, (s + 1) * 64)
                    nc.tensor.matmul(
                        sb[:, s * QW:(s + 1) * QW],
                        kt[p][esl, tsl], qt[p][esl, qoff:qoff + QW],
                        start=True, stop=True,
                    )
                ptt = ptp.tile([128, 2 * QW], bf16, tag="p", name="pt")
                nc.scalar.activation(ptt[:], sb[:], Exp)
                pts[i] = ptt
                del spair[i]
            j = i - SKEW
            if 0 <= j < NSTEP:
                p, q4, t = j // 64, (j // 16) % 4, j % 16
                k8 = j // 16
                ptt = pts.pop(j)
                for s in range(2):
                    h = 2 * p + s
                    nc.tensor.matmul(
                        rep_ps[k8][s][:],
                        vaug[t][:, h, 0:65], ptt[:, s * QW:(s + 1) * QW],
                        start=(t == 0), stop=(t == NKT - 1),
                    )
                if t == NKT - 1:
                    rr = []
                    for s in range(2):
                        r = smp.tile([65, QW], bf16, tag=f"rr{s}", name=f"rr{s}")
                        nc.vector.tensor_copy(r[:], rep_ps[k8][s][:])
                        rr.append(r)
                    rrawb[k8] = rr
                    del rep_ps[k8]
            for fn in hooks.get(i, ()):
                fn()
        # --- tail: last quarter normalize + final output tiles
        norm_chain(7, rrawb.pop(7))
        for t in range(12, 16):
            outproj_tile(t)

    nc.compile()
    return nc


def _prep_core_inputs(c, x1, x2, v, Wq, Wk, Wv, Wo, identf=None, identb=None):
    bf = ml_dtypes.bfloat16
    b, g = c // 2, c % 2
    hs = slice(g * HPC, (g + 1) * HPC)
    wq = (Wq[hs] * (1.0 / np.sqrt(E))).astype(np.float32)   # fold 1/sqrt(E)
    wk, wv, wo = Wk[hs], Wv[hs], Wo[hs]

    def t_pack_pair(w):
        # [4,E,D] -> per pair p: concat(w[2p].T, w[2p+1].T, axis=1) [D,128]
        out = np.empty((2, KT, 128, 128), bf)
        for p in range(2):
            m = np.concatenate([w[2 * p].T, w[2 * p + 1].T], axis=1)  # [D,128]
            out[p] = m.reshape(KT, 128, 128).astype(bf)
        return out

    xq = np.ascontiguousarray(x2[b].T).astype(bf).reshape(KT, 128, N)
    xk = np.ascontiguousarray(x1[b].T).astype(bf).reshape(KT, 128, N)
    xv = np.ascontiguousarray(v[b].T).astype(bf).reshape(KT, 128, N)
    wvT = np.concatenate([wv[h].T for h in range(HPC)], axis=1)  # [D, 256]
    # pair-stacked output weights: [2, 128, D], rows = [E of h=2p; E of 2p+1]
    woT = np.stack([
        np.concatenate([wo[2 * p].T, wo[2 * p + 1].T], axis=0)
        for p in range(2)
    ])
    return {
        "xqT": xq, "xkT": xk, "vT": xv,
        "wqT": t_pack_pair(wq), "wkT": t_pack_pair(wk),
        "wvT": np.ascontiguousarray(wvT).astype(bf).reshape(KT, 128, HPC * E),
        "woT": woT.astype(bf),
    }


def kernel(**inputs):
    from concourse.bass_utils import run_bass_kernel_spmd

    x1 = np.asarray(inputs["x1"], np.float32)
    x2 = np.asarray(inputs["x2"], np.float32)
    v = np.asarray(inputs["v"], np.float32)
    Wq = np.asarray(inputs["Wq"], np.float32)
    Wk = np.asarray(inputs["Wk"], np.float32)
    Wv = np.asarray(inputs["Wv"], np.float32)
    Wo = np.asarray(inputs["Wo"], np.float32)

    if "nc" not in _CACHE:
        _CACHE["nc"] = _build()
    nc = _CACHE["nc"]

    in_maps = [
        _prep_core_inputs(c, x1, x2, v, Wq, Wk, Wv, Wo)
        for c in range(N_CORES)
    ]
    res = run_bass_kernel_spmd(nc, in_maps, list(range(N_CORES)))
    out = np.empty((B, N, D), np.float32)
    for b in range(B):
        out[b] = (
            res.results[2 * b]["outp"].reshape(N, D).astype(np.float32)
            + res.results[2 * b + 1]["outp"].reshape(N, D).astype(np.float32)
        )
    return out
